# revision 5
# baseline (speedup 1.0000x reference)
"""Trainium2 Bass kernel for InterpretableMultiHead attention.

Reference computation (B=2, S=2048, D=512, H=8, DK=64):
    v      = values @ Wv                  [B,S,DK]   (shared across heads)
    q_h    = queries @ Wq[h]              [B,H,S,DK]
    k_h    = keys @ Wk[h]                 [B,H,S,DK]
    scores = q @ k^T / sqrt(DK)           [B,H,S,S]
    attn   = softmax(scores, -1)
    heads  = attn @ v                     [B,H,S,DK]
    out    = mean_h(heads) @ Wh           [B,S,D]
    returns (out, attn[:, -1])            (attention weights of last head)

Sharding: 8 cores = 2 batches x 4 query-row blocks of 512 rows. Each core
computes all 8 heads for its row block; no collectives needed. Host
concatenates the per-core output blocks.

On-chip dataflow (per core), all matmuls in fp16 with fp32 PSUM accumulation:
  - activations are fed pre-transposed from the host (d on partitions)
  - qT/kT computed per head-PAIR so each [128,*] PSUM tile holds two heads
    (partitions 0-63 = even head, 64-127 = odd head)
  - scoresT[t,s] per head via row-packed matmuls (K=64, tile_position 0/64)
  - exp on ACT (scale=1/8 folded in; no max-subtraction needed: |scores|<~2)
  - heads^T = v_aug^T-style matmul with a ones column appended to v, so the
    softmax row-sums fall out of the same accumulating matmul (m=65)
  - per-head normalization via reciprocal + PE outer-product broadcast
  - mean over heads folded as (Wh/8) on host; final out = meanT^T @ Wh
  - attn output for head 7: PE-transpose of its exp tiles + DVE accumulate
    for row sums, normalize, DMA out.
"""

import numpy as np

from contextlib import ExitStack

from concourse import bass, bacc, tile, mybir
from concourse.bass_utils import run_bass_kernel_spmd

B, S, D, H = 2, 2048, 512, 8
DK = D // H          # 64
SQ = 512             # query rows per core
NCORES = 8
P = 128
ND = D // P          # 4 d-chunks
NT = S // P          # 16 t-chunks
NTS = S // 512       # 4 t-slices of 512
NSC = SQ // P        # 4 s-chunks
NPAIR = H // 2       # 4 head pairs

f16 = mybir.dt.float16
f32 = mybir.dt.float32
EXP = mybir.ActivationFunctionType.Exp
MULT = mybir.AluOpType.mult
ADD = mybir.AluOpType.add
AXX = mybir.AxisListType.X

_CACHE = {}


def build_program():
    nc = bacc.Bacc(
        "TRN2",
        target_bir_lowering=False,
        debug=False,
        enable_asserts=False,
        num_devices=NCORES,
    )

    qT_d = nc.dram_tensor("qT", [D, SQ], f16, kind="ExternalInput")
    kT_d = nc.dram_tensor("kT", [D, S], f16, kind="ExternalInput")
    vT_d = nc.dram_tensor("vT", [D, S], f16, kind="ExternalInput")
    wq_d = nc.dram_tensor("wq", [D, H * DK], f16, kind="ExternalInput")
    wk_d = nc.dram_tensor("wk", [D, H * DK], f16, kind="ExternalInput")
    wv_d = nc.dram_tensor("wv", [D, DK], f16, kind="ExternalInput")
    wh_d = nc.dram_tensor("wh", [DK, D], f16, kind="ExternalInput")
    id_d = nc.dram_tensor("ident", [P, P], f16, kind="ExternalInput")
    out_d = nc.dram_tensor("out", [SQ, D], f32, kind="ExternalOutput")
    attn_d = nc.dram_tensor("attn7", [SQ, S], f32, kind="ExternalOutput")

    with tile.TileContext(nc) as tc, ExitStack() as ctx:
        const = ctx.enter_context(tc.tile_pool(name="const", bufs=1))
        expp = ctx.enter_context(tc.tile_pool(name="expp", bufs=10))
        small = ctx.enter_context(tc.tile_pool(name="small", bufs=4))
        a7p = ctx.enter_context(tc.tile_pool(name="a7p", bufs=3))
        obp = ctx.enter_context(tc.tile_pool(name="obp", bufs=2))
        psA = ctx.enter_context(tc.tile_pool(name="psA", bufs=4, space="PSUM"))
        psB = ctx.enter_context(tc.tile_pool(name="psB", bufs=3, space="PSUM"))
        psT = ctx.enter_context(tc.tile_pool(name="psT", bufs=1, space="PSUM"))

        # ---- persistent SBUF tiles ----
        qt_sb = const.tile([P, ND, SQ], f16, tag="qt")
        kt_sb = const.tile([P, ND, S], f16, tag="kt")
        vt_sb = const.tile([P, ND, S], f16, tag="vt")
        wq_sb = const.tile([P, ND, H * DK], f16, tag="wq")
        wk_sb = const.tile([P, ND, H * DK], f16, tag="wk")
        wv_sb = const.tile([P, ND, DK], f16, tag="wv")
        wh_sb = const.tile([DK, D], f16, tag="wh")
        id_sb = const.tile([P, P], f16, tag="id")
        qTp = const.tile([P, NPAIR, SQ], f16, tag="qTp")    # pair layout
        kTp = const.tile([P, NPAIR, S], f16, tag="kTp")     # pair layout
        vaug = const.tile([P, NT, DK + 1], f16, tag="vaug")
        exp7 = const.tile([P, NT, SQ], f16, tag="exp7")
        hn = const.tile([DK, SQ, H], f32, tag="hn")
        mean32 = const.tile([DK, SQ], f32, tag="mean32")
        meanh = const.tile([DK, SQ], f16, tag="meanh")
        ones64 = const.tile([1, DK], f16, tag="ones64")

        # ---- DMA loads ----
        nc.sync.dma_start(id_sb[:, :], id_d[:, :])
        nc.vector.memset(ones64[:, :], 1.0)
        for di in range(ND):
            nc.sync.dma_start(wq_sb[:, di, :], wq_d[di * P:(di + 1) * P, :])
            nc.sync.dma_start(wk_sb[:, di, :], wk_d[di * P:(di + 1) * P, :])
            nc.sync.dma_start(wv_sb[:, di, :], wv_d[di * P:(di + 1) * P, :])
            nc.sync.dma_start(qt_sb[:, di, :], qT_d[di * P:(di + 1) * P, :])
        for di in range(ND):
            nc.sync.dma_start(kt_sb[:, di, :], kT_d[di * P:(di + 1) * P, :])
        for di in range(ND):
            nc.sync.dma_start(vt_sb[:, di, :], vT_d[di * P:(di + 1) * P, :])
        nc.sync.dma_start(wh_sb[:, :], wh_d[:, :])

        # ---- projections ----
        # v[t, dk] = sum_d values[t, d] Wv[d, dk]; lhsT = vT chunk (stationary)
        for c in range(NT):
            pv = psB.tile([P, DK], f32, tag="hd")
            for di in range(ND):
                nc.tensor.matmul(
                    pv[:, :],
                    vt_sb[:, di, c * P:(c + 1) * P],
                    wv_sb[:, di, :],
                    start=(di == 0),
                    stop=(di == ND - 1),
                )
            nc.vector.tensor_copy(vaug[:, c, 0:DK], pv[:, :])
            nc.vector.memset(vaug[:, c, DK:DK + 1], 1.0)

        # qT / kT per head pair: lhsT = W chunk [128d, 128(2 heads x dk)]
        pair_order = [3, 0, 1, 2]  # head 7's pair first so attn output overlaps
        for p in pair_order:
            pq = psA.tile([P, SQ], f32, tag="mm")
            for di in range(ND):
                nc.tensor.matmul(
                    pq[:, :],
                    wq_sb[:, di, p * P:(p + 1) * P],
                    qt_sb[:, di, :],
                    start=(di == 0),
                    stop=(di == ND - 1),
                )
            nc.vector.tensor_copy(qTp[:, p, :], pq[:, :])
            for n in range(NTS):
                pk = psA.tile([P, 512], f32, tag="mm")
                for di in range(ND):
                    nc.tensor.matmul(
                        pk[:, :],
                        wk_sb[:, di, p * P:(p + 1) * P],
                        kt_sb[:, di, n * 512:(n + 1) * 512],
                        start=(di == 0),
                        stop=(di == ND - 1),
                    )
                nc.vector.tensor_copy(kTp[:, p, n * 512:(n + 1) * 512], pk[:, :])

        # ---- attention per head pair ----
        for p in pair_order:
            h0, h1 = 2 * p, 2 * p + 1
            ph0 = psB.tile([DK + 1, SQ], f32, tag="hd")
            ph1 = psB.tile([DK + 1, SQ], f32, tag="hd")
            for c in range(NT):
                s0 = psA.tile([P, SQ], f32, tag="mm")
                s1 = psA.tile([P, SQ], f32, tag="mm")
                nc.tensor.matmul(
                    s0[:, :],
                    kTp[0:DK, p, c * P:(c + 1) * P],
                    qTp[0:DK, p, :],
                    start=True, stop=True,
                    tile_position=(0, 0),
                )
                nc.tensor.matmul(
                    s1[:, :],
                    kTp[DK:P, p, c * P:(c + 1) * P],
                    qTp[DK:P, p, :],
                    start=True, stop=True,
                    tile_position=(64, 0),
                )
                e0 = expp.tile([P, SQ], f16, tag="exp")
                if h1 == 7:
                    e1 = exp7[:, c, :]
                else:
                    e1 = expp.tile([P, SQ], f16, tag="exp")
                nc.scalar.activation(e0[:, :], s0[:, :], EXP, scale=0.125)
                nc.scalar.activation(e1[:, :], s1[:, :], EXP, scale=0.125)
                nc.tensor.matmul(
                    ph0[:, :], vaug[:, c, :], e0[:, :],
                    start=(c == 0), stop=(c == NT - 1),
                )
                nc.tensor.matmul(
                    ph1[:, :], vaug[:, c, :], e1[:, :],
                    start=(c == 0), stop=(c == NT - 1),
                )
            # normalize each head and stash for the mean
            for h, ph in ((h0, ph0), (h1, ph1)):
                rs = small.tile([1, SQ], f32, tag="rs")
                nc.vector.reciprocal(rs[:, :], ph[DK:DK + 1, :])
                rsh = small.tile([1, SQ], f16, tag="rsh")
                nc.vector.tensor_copy(rsh[:, :], rs[:, :])
                rb = psA.tile([DK, SQ], f32, tag="mm")
                nc.tensor.matmul(rb[:, :], ones64[:, :], rsh[:, :],
                                 start=True, stop=True)
                rb_sb = small.tile([DK, SQ], f32, tag="rb_sb")
                nc.vector.tensor_copy(rb_sb[:, :], rb[:, :])
                nc.vector.tensor_mul(hn[:, :, h], ph[0:DK, :], rb_sb[:, :])

        # ---- head-7 attention output: transpose exp7, normalize, store ----
        for sc in range(NSC):
            a7 = a7p.tile([P, S], f32, tag="a7")
            rs7p = small.tile([P, NT], f32, tag="rs7p")
            for c in range(NT):
                tp = psT.tile([P, P], f16, tag="tp")
                nc.tensor.transpose(
                    tp[:, :], exp7[:, c, sc * P:(sc + 1) * P], id_sb[:, :]
                )
                nc.vector.tensor_scalar(
                    out=a7[:, c * P:(c + 1) * P],
                    in0=tp[:, :],
                    scalar1=1.0,
                    scalar2=None,
                    op0=MULT,
                    op1=ADD,
                    accum_out=rs7p[:, c:c + 1],
                )
            rs7 = small.tile([P, 1], f32, tag="rs7")
            nc.vector.tensor_reduce(rs7[:, :], rs7p[:, :], AXX, ADD)
            rc7 = small.tile([P, 1], f32, tag="rc7")
            nc.vector.reciprocal(rc7[:, :], rs7[:, :])
            nc.vector.tensor_scalar_mul(a7[:, :], a7[:, :], rc7[:, :])
            nc.sync.dma_start(attn_d[sc * P:(sc + 1) * P, :], a7[:, :])

        # ---- mean over heads (1/H folded into Wh) + output projection ----
        nc.vector.tensor_reduce(mean32[:, :], hn[:, :, :], AXX, ADD)
        nc.vector.tensor_copy(meanh[:, :], mean32[:, :])
        for sc in range(NSC):
            po = psA.tile([P, D], f32, tag="mm")
            nc.tensor.matmul(po[:, :], meanh[:, sc * P:(sc + 1) * P],
                             wh_sb[:, :], start=True, stop=True)
            ob = obp.tile([P, D], f32, tag="ob")
            nc.vector.tensor_copy(ob[:, :], po[:, :])
            nc.sync.dma_start(out_d[sc * P:(sc + 1) * P, :], ob[:, :])

    nc.compile()
    return nc


def get_program():
    if "nc" not in _CACHE:
        _CACHE["nc"] = build_program()
    return _CACHE["nc"]


def make_in_maps(queries, keys, values, Wv, Wq, Wk, Wh):
    queries = np.asarray(queries, dtype=np.float32)
    keys = np.asarray(keys, dtype=np.float32)
    values = np.asarray(values, dtype=np.float32)
    Wv = np.asarray(Wv, dtype=np.float32)
    Wq = np.asarray(Wq, dtype=np.float32)
    Wk = np.asarray(Wk, dtype=np.float32)
    Wh = np.asarray(Wh, dtype=np.float32)

    # [H, D, DK] -> [D, H*DK] so head-pair p occupies columns p*128:(p+1)*128
    wq_m = np.ascontiguousarray(np.transpose(Wq, (1, 0, 2)).reshape(D, H * DK)).astype(np.float16)
    wk_m = np.ascontiguousarray(np.transpose(Wk, (1, 0, 2)).reshape(D, H * DK)).astype(np.float16)
    wv_m = np.ascontiguousarray(Wv).astype(np.float16)
    wh_m = np.ascontiguousarray(Wh / np.float32(H)).astype(np.float16)
    ident = np.eye(P, dtype=np.float16)

    in_maps = []
    for core in range(NCORES):
        b, j = divmod(core, NCORES // B)
        qT = np.ascontiguousarray(queries[b, j * SQ:(j + 1) * SQ, :].T).astype(np.float16)
        kT = np.ascontiguousarray(keys[b].T).astype(np.float16)
        vT = np.ascontiguousarray(values[b].T).astype(np.float16)
        in_maps.append({
            "qT": qT, "kT": kT, "vT": vT,
            "wq": wq_m, "wk": wk_m, "wv": wv_m, "wh": wh_m,
            "ident": ident,
        })
    return in_maps


def assemble(results):
    out = np.empty((B, S, D), np.float32)
    attn = np.empty((B, S, S), np.float32)
    for core in range(NCORES):
        b, j = divmod(core, NCORES // B)
        out[b, j * SQ:(j + 1) * SQ, :] = results[core]["out"]
        attn[b, j * SQ:(j + 1) * SQ, :] = results[core]["attn7"]
    return out, attn


def kernel(queries, keys, values, Wv, Wq, Wk, Wh, **_ignored):
    nc = get_program()
    in_maps = make_in_maps(queries, keys, values, Wv, Wq, Wk, Wh)
    res = run_bass_kernel_spmd(nc, in_maps, core_ids=list(range(NCORES)))
    return assemble(res.results)
